# revision 13
# baseline (speedup 1.0000x reference)
"""Morphological dilation (max-plus 3x3 depthwise conv) on 8 Trainium2 cores.

out[b,c,y,x] = max_{i,j in 3x3} ( x_pad[b,c,y+i,x+j] + se[c,i,j] ),
x: [16,64,256,256] f32, se: [64,3,3] f32, pad=1 with CVAL=-10000.

Sharding: pure data parallel. Core k takes batches {2k, 2k+1}; the 2*64
(batch,channel) pairs map onto the 128 SBUF partitions, so se[c,i,j] is a
per-partition scalar. Spatial dims live on the free axis; each 3x3 tap is one
fused DVE scalar_tensor_tensor: acc = (x_shifted + se_ij) max acc.

Sync-wait budgets on TRN2 are tiny (TensorScalarPtr and DMA encodings fit ONE
semaphore wait; even the epilogue drain has a small cap), so the program is
arranged to keep every instruction at <=1 wait and the number of distinct
queue semaphores low:
  - the whole per-core x shard lives in ONE persistent SBUF tile; the se load
    and the 5 chunked input loads are serially chained on one forced HWDGE
    queue (loads have no other deps, so the chain wait is their only one),
  - each of the 5 stores gets its own dedicated HWDGE queue (a store's only
    wait is then the DVE semaphore),
  - per block, two 1-element acc memsets act as wait-absorbers: one waits for
    the input chunk the block needs (next block's load covers this block's
    bottom halo, and the serial chain covers everything earlier incl. the se
    load), one waits for the store whose acc slot the block reuses. The taps
    follow them by region dependency and only ever carry their own-engine
    (DVE) wait.
"""

import os
import numpy as np

B, C, H, W = 16, 64, 256, 256
NCORES = 8
P = 128  # partitions = (B // NCORES) * C
CVAL = -10000.0
KH = KW = 3

# f16: DVE 2x packed mode, ~5e-4 scale-relative error.
# f32: bit-exact vs reference, 1x DVE rate.
_DTYPE = os.environ.get("DILATION_DTYPE", "f16")

_nc_cache = {}
LAST_RESULTS = None  # BassKernelResults of the most recent run (for profiling)

# instruction name -> forced HWDGE queue index (consulted by the patched
# TileClockTick._assign_tick during scheduling)
_FORCED_HW_QUEUE = {}
_ASSIGN_PATCHED = False


def _patch_queue_assignment():
    global _ASSIGN_PATCHED
    if _ASSIGN_PATCHED:
        return
    import concourse.tile_sem_assignment as tsa

    orig = tsa.TileClockTick._assign_tick

    def _assign_tick(self, inst):
        forced = _FORCED_HW_QUEUE.get(getattr(inst, "name", None))
        if forced is None:
            return orig(self, inst)
        save = self.next_hw_dma_idx
        self.next_hw_dma_idx = forced
        try:
            return orig(self, inst)
        finally:
            self.next_hw_dma_idx = save

    tsa.TileClockTick._assign_tick = _assign_tick
    _ASSIGN_PATCHED = True


def _block_sizes(h: int, nblocks: int):
    base = h // nblocks
    rem = h - base * nblocks
    return [base + (1 if i < rem else 0) for i in range(nblocks)]


def _build(dtype_tag: str, h: int = H, nblocks: int = 5):
    import concourse.bass as bass
    import concourse.mybir as mybir
    from concourse.tile import TileContext, add_dep_helper

    _patch_queue_assignment()
    _FORCED_HW_QUEUE.clear()

    dt = mybir.dt.float16 if dtype_tag == "f16" else mybir.dt.float32
    add = mybir.AluOpType.add
    vmax = mybir.AluOpType.max

    nc = bass.Bass(trn_type="TRN2")
    x_d = nc.declare_dram_parameter("x", [P, h, W], dt, isOutput=False)
    se_d = nc.declare_dram_parameter("sep", [P, KH * KW], mybir.dt.float32, isOutput=False)
    out_d = nc.declare_dram_parameter("out", [P, h, W], dt, isOutput=True)

    blocks = _block_sizes(h, nblocks)

    with TileContext(nc) as tc:
        with (
            tc.tile_pool(name="const", bufs=1) as cpool,
            tc.tile_pool(name="xp", bufs=1) as xpool,
            tc.tile_pool(name="accp", bufs=2) as apool,
        ):
            se_t = cpool.tile([P, KH * KW], mybir.dt.float32)
            se_dma = nc.sync.dma_start(out=se_t[:], in_=se_d[:])
            _FORCED_HW_QUEUE[se_dma.ins.name] = 0
            # one fresh 1-element scratch per block: a memset on it can carry
            # a single DMA wait with no other dependencies attached
            scr = [
                cpool.tile([P, 1], dt, name=f"scr{b}") for b in range(nblocks)
            ]

            # One persistent padded-x tile: xt row t = padded-input row t,
            # col c = padded-input col c.
            xt = xpool.tile([P, h + 2, W + 2], dt)
            # pads (set once; disjoint from every load region)
            nc.vector.memset(xt[:, :, 0:1], CVAL)
            nc.vector.memset(xt[:, :, W + 1 : W + 2], CVAL)
            nc.vector.memset(xt[:, 0:1, :], CVAL)
            nc.vector.memset(xt[:, h + 1 : h + 2, :], CVAL)

            # Chunked loads, serially chained on HWDGE queue 0: waiting for
            # load k implies every earlier load (and the se load) finished.
            load_dmas = []
            y0 = 0
            for blk, rows in enumerate(blocks):
                ld = nc.sync.dma_start(
                    out=xt[:, y0 + 1 : y0 + rows + 1, 1 : W + 1],
                    in_=x_d[:, y0 : y0 + rows, :],
                )
                _FORCED_HW_QUEUE[ld.ins.name] = 0
                load_dmas.append(ld)
                y0 += rows

            out_dmas = []
            last_taps = []
            y0 = 0
            for blk, rows in enumerate(blocks):
                acc = apool.tile([P, rows, W], dt)
                # Wait-absorbers (first acc writers, 1-element each).
                # m2 (store whose acc slot this block reuses) must be the
                # FIRST acc writer so it carries the bundled slot-release
                # dependency; the ordering edge behind the releasing block's
                # last tap puts it late enough in the DVE stream that the
                # release's DVE component is elided, leaving one DMA wait.
                if blk >= 2:
                    # ma: sole carrier of the store-DMA wait (scratch write,
                    # no other deps). m2: first acc writer; by the time it
                    # runs the DVE has observed the store sem via ma, so the
                    # bundled slot-release costs it only its DVE wait.
                    ma = nc.vector.memset(scr[blk][:, 0:1], 0.0)
                    add_dep_helper(
                        ma.ins, out_dmas[blk - 2].ins, reason="absorb acc-slot WAR"
                    )
                    m2 = nc.vector.memset(acc[:, 0:1, 1:2], 0.0)
                    add_dep_helper(
                        m2.ins, ma.ins, sync=False, reason="after the WAR absorber"
                    )
                # m1 waits for the deepest input chunk this block reads.
                m1 = nc.vector.memset(acc[:, 0:1, 0:1], 0.0)
                add_dep_helper(
                    m1.ins,
                    load_dmas[min(blk + 1, nblocks - 1)].ins,
                    reason="absorb input-chunk wait",
                )
                first = True
                for i in range(KH):
                    for j in range(KW):
                        xin = xt[:, y0 + i : y0 + i + rows, j : j + W]
                        sca = se_t[:, 3 * i + j : 3 * i + j + 1]
                        if first:
                            nc.vector.tensor_scalar(acc[:], xin, sca, None, add)
                            first = False
                        else:
                            lt = nc.vector.scalar_tensor_tensor(
                                acc[:], xin, sca, acc[:], add, vmax
                            )
                last_taps.append(lt)
                od = nc.sync.dma_start(out=out_d[:, y0 : y0 + rows, :], in_=acc[:])
                _FORCED_HW_QUEUE[od.ins.name] = 1 + blk  # dedicated queue
                out_dmas.append(od)
                y0 += rows

    _split_excess_waits(nc, mybir)
    return nc


def _split_excess_waits(nc, mybir, max_waits: int = 1):
    """Walrus's per-encoding sync-wait slots are scarce (1 for most ops used
    here). Hoist all but `max_waits` waits of any instruction onto freshly
    inserted same-engine Drain instructions placed right before it."""
    n = 0
    for bb in nc.main_func.blocks:
        insts = bb.instructions
        i = 0
        while i < len(insts):
            ins = insts[i]
            si = ins.sync_info
            if si is not None and len(si.on_wait) > max_waits:
                waits = list(si.on_wait)
                keep = waits[-max_waits:]
                spill = waits[:-max_waits]
                new_insts = []
                for w in spill:
                    d = mybir.InstDrain(name=f"wsplit-{n}", ins=[], outs=[])
                    n += 1
                    d.engine = ins.engine
                    d.sync_info = mybir.SyncInfo(on_wait=[w], on_update=[])
                    new_insts.append(d)
                ins.sync_info = mybir.SyncInfo(
                    on_wait=keep, on_update=list(si.on_update)
                )
                insts[i:i] = new_insts
                i += len(new_insts)
            i += 1
        bb.instructions = insts


def _get_nc():
    key = (_DTYPE,)
    if key not in _nc_cache:
        _nc_cache[key] = _build(_DTYPE)
    return _nc_cache[key]


def kernel(x: np.ndarray, se: np.ndarray) -> np.ndarray:
    global LAST_RESULTS
    from concourse.bass_utils import run_bass_kernel_spmd

    np_dt = np.float16 if _DTYPE == "f16" else np.float32
    x = np.asarray(x)
    se = np.asarray(se)
    xs = np.ascontiguousarray(x).reshape(NCORES, P, H, W).astype(np_dt)
    sep = np.ascontiguousarray(
        np.tile(np.asarray(se, np.float32).reshape(C, KH * KW), (P // C, 1))
    )

    nc = _get_nc()
    in_maps = [{"x": xs[k], "sep": sep} for k in range(NCORES)]
    trace = bool(os.environ.get("DILATION_TRACE"))
    kwargs = {}
    if trace:
        kwargs["trace"] = True
        tmpdir = os.environ.get("DILATION_TRACE_DIR")
        if tmpdir:
            kwargs["tmpdir"] = tmpdir
    res = run_bass_kernel_spmd(nc, in_maps, list(range(NCORES)), **kwargs)
    LAST_RESULTS = res
    out = np.stack([res.results[k]["out"] for k in range(NCORES)])
    return out.reshape(B, C, H, W).astype(np.float32)


# revision 17
# speedup vs baseline: 1.4051x; 1.4051x over previous
"""Morphological dilation (max-plus 3x3 depthwise conv) on 8 Trainium2 cores.

out[b,c,y,x] = max_{i,j in 3x3} ( x_pad[b,c,y+i,x+j] + se[c,i,j] ),
x: [16,64,256,256] f32, se: [64,3,3] f32, pad=1 with CVAL=-10000.

Sharding: pure data parallel. Core k takes batches {2k, 2k+1}; the 2*64
(batch,channel) pairs map onto the 128 SBUF partitions, so se[c,i,j] is a
per-partition scalar. Spatial dims live on the free axis.

Measured DVE modes (fp16, 0.96 GHz): scalar_tensor_tensor is 1x only;
tensor_scalar is 4x when 4B-aligned (2x at odd offsets); tensor_tensor is 2x.
ACT (1.2 GHz) does Identity(in + per-partition bias) at 1x and is otherwise
idle. So each tap is add+max with the adds split between engines:
  - 3 taps (j=0, 4B-aligned): DVE tensor_scalar add (4x) + tensor_tensor max (2x)
  - 6 taps (j=1 odd, j=2): ACT Identity+bias add into ping-pong tmp tiles,
    DVE tensor_tensor max (2x)
DVE ~41us/block vs ACT ~43us/block -> balanced pipeline.

Sync-wait budgets are 1 per instruction for every compute/DMA encoding used
here, so cross-engine handoffs go through 1-element "gate" ops that carry the
single foreign-semaphore wait (the consumer then only needs its own-engine
wait): DVE memset gates before each TT that reads an ACT tmp, ACT 1-element
Identity gates for tmp-slot reuse and input-chunk waits. x is fully
SBUF-resident (one persistent tile, 5 chunked loads serially chained on one
HWDGE queue); the 8 per-block stores use 7 dedicated HWDGE queues + 1 SWDGE
queue so no store ever chains. A post-pass splits any remaining multi-wait
instruction (the framework epilogue drain) into single-wait drains.
"""

import os
import numpy as np

B, C, H, W = 16, 64, 256, 256
NCORES = 8
P = 128  # partitions = (B // NCORES) * C
CVAL = -10000.0
KH = KW = 3

_DTYPE = os.environ.get("DILATION_DTYPE", "f16")

_nc_cache = {}
LAST_RESULTS = None  # BassKernelResults of the most recent run (for profiling)

# instruction name -> forced HWDGE queue index (consulted by the patched
# TileClockTick._assign_tick during scheduling)
_FORCED_HW_QUEUE = {}
_ASSIGN_PATCHED = False

# taps: (i, j) with per-tap scalar index t = 3*i + j
_DVE_TAPS = [(0, 0), (1, 0), (2, 0)]  # j=0 -> 4B-aligned reads
_ACT_TAPS = [(0, 1), (1, 1), (2, 1), (0, 2), (1, 2), (2, 2)]


def _patch_queue_assignment():
    global _ASSIGN_PATCHED
    if _ASSIGN_PATCHED:
        return
    import concourse.tile_sem_assignment as tsa

    orig = tsa.TileClockTick._assign_tick

    def _assign_tick(self, inst):
        forced = _FORCED_HW_QUEUE.get(getattr(inst, "name", None))
        if forced is None:
            return orig(self, inst)
        save = self.next_hw_dma_idx
        self.next_hw_dma_idx = forced
        try:
            return orig(self, inst)
        finally:
            self.next_hw_dma_idx = save

    tsa.TileClockTick._assign_tick = _assign_tick
    _ASSIGN_PATCHED = True


def _split_excess_waits(nc, mybir, max_waits: int = 1):
    """Walrus's per-encoding sync-wait slots are scarce (1 for most ops used
    here). Hoist all but `max_waits` waits of any instruction onto freshly
    inserted same-engine Drain instructions placed right before it."""
    n = 0
    for bb in nc.main_func.blocks:
        insts = bb.instructions
        i = 0
        while i < len(insts):
            ins = insts[i]
            si = ins.sync_info
            if si is not None and len(si.on_wait) > max_waits:
                waits = list(si.on_wait)
                keep = waits[-max_waits:]
                spill = waits[:-max_waits]
                new_insts = []
                for w in spill:
                    d = mybir.InstDrain(name=f"wsplit-{n}", ins=[], outs=[])
                    n += 1
                    d.engine = ins.engine
                    d.sync_info = mybir.SyncInfo(on_wait=[w], on_update=[])
                    new_insts.append(d)
                ins.sync_info = mybir.SyncInfo(
                    on_wait=keep, on_update=list(si.on_update)
                )
                insts[i:i] = new_insts
                i += len(new_insts)
            i += 1
        bb.instructions = insts


def _block_sizes(h: int, nblocks: int):
    base = h // nblocks
    rem = h - base * nblocks
    return [base + (1 if i < rem else 0) for i in range(nblocks)]


def _build(dtype_tag: str, h: int = H, nblocks: int = 10, nloads: int = 5, split_waits: bool = True):
    import concourse.bass as bass
    import concourse.mybir as mybir
    from concourse.tile import TileContext, add_dep_helper

    _patch_queue_assignment()
    _FORCED_HW_QUEUE.clear()

    assert dtype_tag == "f16", "v4 layout is fp16-only"
    dt = mybir.dt.float16
    f32 = mybir.dt.float32
    add = mybir.AluOpType.add
    vmax = mybir.AluOpType.max
    ident = mybir.ActivationFunctionType.Identity

    nc = bass.Bass(trn_type="TRN2", num_swdge_queues=3)
    x_d = nc.declare_dram_parameter("x", [P, h, W], dt, isOutput=False)
    se_d = nc.declare_dram_parameter("sep", [P, KH * KW], f32, isOutput=False)
    out_d = nc.declare_dram_parameter("out", [P, h, W], dt, isOutput=True)

    blocks = _block_sizes(h, nblocks)
    loads = _block_sizes(h, nloads)

    with TileContext(nc) as tc:
        with (
            tc.tile_pool(name="const", bufs=1) as cpool,
            tc.tile_pool(name="xp", bufs=1) as xpool,
            tc.tile_pool(name="accp", bufs=2) as apool,
            tc.tile_pool(name="tmpp", bufs=1) as tpool,
        ):
            se_t = cpool.tile([P, KH * KW], f32, name="se_t")
            se_dma = nc.sync.dma_start(out=se_t[:], in_=se_d[:])
            _FORCED_HW_QUEUE[se_dma.ins.name] = 0

            # One persistent padded-x tile: xt row t = padded-input row t.
            xt = xpool.tile([P, h + 2, W + 2], dt, name="xt")
            nc.vector.memset(xt[:, :, 0:1], CVAL)
            nc.vector.memset(xt[:, :, W + 1 : W + 2], CVAL)
            nc.vector.memset(xt[:, 0:1, :], CVAL)
            nc.vector.memset(xt[:, h + 1 : h + 2, :], CVAL)

            # Chunked loads, serially chained on HWDGE queue 0.
            load_dmas = []
            load_top = []  # last loaded input row (exclusive) per chunk
            y0 = 0
            for rows in loads:
                ld = nc.sync.dma_start(
                    out=xt[:, y0 + 1 : y0 + rows + 1, 1 : W + 1],
                    in_=x_d[:, y0 : y0 + rows, :],
                )
                _FORCED_HW_QUEUE[ld.ins.name] = 0
                load_dmas.append(ld)
                y0 += rows
                load_top.append(y0)

            # scratch tiles for gates (tiny 1-element targets)
            dve_scr = cpool.tile([P, 2 * nblocks], dt, name="dve_scr")
            act_scr = cpool.tile([P, 4], dt, name="act_scr")
            act_src = cpool.tile([P, 1], dt, name="act_src")
            nc.vector.memset(act_src[:], 0.0)

            # ping-pong tmp tiles for the ACT adds
            maxrows = max(blocks)
            tmps = [tpool.tile([P, maxrows, W], dt, name=f"tmp{i}") for i in range(2)]
            tmp_reader = [None, None]  # TT that last read each tmp slot

            out_dmas = []
            y0 = 0
            tmp_idx = 0
            for blk, rows in enumerate(blocks):
                # deepest load chunk this block needs (bottom halo row is
                # input row y0+rows; the queue-0 chain covers earlier chunks)
                need_top = min(y0 + rows + 1, h)
                ldi = next(i for i, top in enumerate(load_top) if top >= need_top)

                acc = apool.tile([P, rows, W], dt, name="acc")
                # DVE-side gates: gw absorbs the store whose acc slot this
                # block reuses, gx the input-chunk wait.
                if blk >= 2:
                    gw = nc.vector.memset(dve_scr[:, 2 * blk + 1 : 2 * blk + 2], 0.0)
                    add_dep_helper(gw.ins, out_dmas[blk - 2].ins, reason="acc WAR")
                gx = nc.vector.memset(dve_scr[:, 2 * blk : 2 * blk + 1], 0.0)
                add_dep_helper(gx.ins, load_dmas[ldi].ins, reason="input chunk")
                # ACT-side gate for the input chunk
                ga = nc.scalar.activation(
                    act_scr[:, 0:1], act_src[:, 0:1], ident, bias=se_t[:, 0:1]
                )
                add_dep_helper(ga.ins, load_dmas[ldi].ins, reason="input chunk/ACT")

                def act_add(tap):
                    """Emit one ACT add into the next ping-pong tmp slot,
                    gated on the TT that last read that slot."""
                    nonlocal tmp_idx
                    t_i, t_j = tap
                    ti = tmp_idx % 2
                    tmp_idx += 1
                    sidx = 3 * t_i + t_j
                    if tmp_reader[ti] is not None:
                        gt = nc.scalar.activation(
                            act_scr[:, 1:2], act_src[:, 0:1], ident,
                            bias=se_t[:, 0:1],
                        )
                        add_dep_helper(
                            gt.ins, tmp_reader[ti].ins, reason="tmp WAR gate"
                        )
                    a = nc.scalar.activation(
                        tmps[ti][:, 0:rows, :],
                        xt[:, y0 + t_i : y0 + t_i + rows, t_j : t_j + W],
                        ident,
                        bias=se_t[:, sidx : sidx + 1],
                    )
                    return ti, a

                def act_fold(ti, a):
                    """DVE TT max over a finished ACT tmp (gate carries the
                    cross-engine wait)."""
                    gm = nc.vector.memset(dve_scr[:, 2 * blk : 2 * blk + 1], 0.0)
                    add_dep_helper(gm.ins, a.ins, reason="ACT tmp ready")
                    tt = nc.vector.tensor_tensor(
                        acc[:], acc[:], tmps[ti][:, 0:rows, :], vmax
                    )
                    tmp_reader[ti] = tt

                # Software pipeline: keep both tmp slots busy - emit the add
                # for tap t+2 only after the TT for tap t exists.
                pending = [act_add(_ACT_TAPS[0]), act_add(_ACT_TAPS[1])]
                nxt = 2

                # DVE-only taps: aligned TS init + 2x (TS add -> tmp_d, TT max)
                (i0, j0), *dve_rest = _DVE_TAPS
                nc.vector.tensor_scalar(
                    acc[:],
                    xt[:, y0 + i0 : y0 + i0 + rows, j0 : j0 + W],
                    se_t[:, 3 * i0 + j0 : 3 * i0 + j0 + 1],
                    None,
                    add,
                )
                tmp_d = tpool.tile([P, maxrows, W], dt, name="tmp_d")
                for t_i, t_j in dve_rest:
                    sidx = 3 * t_i + t_j
                    nc.vector.tensor_scalar(
                        tmp_d[:, 0:rows, :],
                        xt[:, y0 + t_i : y0 + t_i + rows, t_j : t_j + W],
                        se_t[:, sidx : sidx + 1],
                        None,
                        add,
                    )
                    nc.vector.tensor_tensor(acc[:], acc[:], tmp_d[:, 0:rows, :], vmax)
                # fold ACT tmps, refilling each slot as it frees up
                while pending:
                    ti, a = pending.pop(0)
                    act_fold(ti, a)
                    if nxt < len(_ACT_TAPS):
                        pending.append(act_add(_ACT_TAPS[nxt]))
                        nxt += 1

                oeng = nc.sync if blk < 7 else nc.gpsimd
                od = oeng.dma_start(out=out_d[:, y0 : y0 + rows, :], in_=acc[:])
                if blk < 7:
                    _FORCED_HW_QUEUE[od.ins.name] = 1 + blk
                out_dmas.append(od)
                y0 += rows

    if split_waits:
        _split_excess_waits(nc, mybir)
    return nc


def _get_nc():
    key = (_DTYPE,)
    if key not in _nc_cache:
        _nc_cache[key] = _build(_DTYPE)
    return _nc_cache[key]


def kernel(x: np.ndarray, se: np.ndarray) -> np.ndarray:
    global LAST_RESULTS
    from concourse.bass_utils import run_bass_kernel_spmd

    np_dt = np.float16 if _DTYPE == "f16" else np.float32
    x = np.asarray(x)
    se = np.asarray(se)
    xs = np.ascontiguousarray(x).reshape(NCORES, P, H, W).astype(np_dt)
    sep = np.ascontiguousarray(
        np.tile(np.asarray(se, np.float32).reshape(C, KH * KW), (P // C, 1))
    )

    nc = _get_nc()
    in_maps = [{"x": xs[k], "sep": sep} for k in range(NCORES)]
    trace = bool(os.environ.get("DILATION_TRACE"))
    kwargs = {}
    if trace:
        kwargs["trace"] = True
        tmpdir = os.environ.get("DILATION_TRACE_DIR")
        if tmpdir:
            kwargs["tmpdir"] = tmpdir
    res = run_bass_kernel_spmd(nc, in_maps, list(range(NCORES)), **kwargs)
    LAST_RESULTS = res
    out = np.stack([res.results[k]["out"] for k in range(NCORES)])
    return out.reshape(B, C, H, W).astype(np.float32)
